# revision 1
# baseline (speedup 1.0000x reference)
"""DeFeat distillation loss on 8 Trainium2 NeuronCores (Bass/Tile).

Data-parallel over the batch dim (B=8 -> 1 batch element per core).
Per core, features are viewed as [C=256, H*W] and streamed in column
blocks (levels 2-4 are fused into one multi-segment block so the DMA
stream never drains on the small levels). Per 512-column matmul tile:
  psum = W @ feat_s                      [TensorE fp32r]
  d    = (feat_t - b) - psum   (bf16)    [VectorE fused, frees psum fast]
  dd   = d^2                   (bf16)    [ScalarE Square, segment-wide]
  q    = ones^T @ dd           (psum)    [TensorE bf16: column sums]
  qcat[tile]  = q (bf16 staging row)     [ScalarE copy]
The mask depends only on the column, so the masked sum factors:
  s_gt = sum_n m[n] * q[n],  s_tot = sum_n q[n].
The host rasterizes the masks, takes the per-core q vectors (85KB each)
and finishes both dot products in float64, then applies sqrt + weights.
"""

import os
import sys

for _p in ("/opt/trn_rl_repo", os.path.expanduser("~/.axon_site/_ro/trn_rl_repo")):
    if os.path.isdir(_p) and _p not in sys.path:
        sys.path.insert(0, _p)

import numpy as np

WEIGHT_GT = 0.004
WEIGHT_BG = 0.0002
STRIDES = (8, 16, 32, 64, 128)
SIZES = (128, 64, 32, 16, 8)
HWS = tuple(s * s for s in SIZES)          # (16384, 4096, 1024, 256, 64)
B, C, NBOX = 8, 256, 16
N_CORES = 8
TILE_N = 512                               # matmul free-dim tile
N_LEVELS = 5
MASK_LEN = sum(HWS)                        # 21824
MASK_OFF = tuple(sum(HWS[:i]) for i in range(N_LEVELS))

# Blocks: list of segment lists [(lvl, c0, w), ...]. First/last blocks are
# narrow so compute starts early and finishes quickly; the small levels sit
# mid-stream fused into one block.
BLOCKS = [
    [(0, 0, 1024)], [(0, 1024, 1024)],
    [(0, 2048, 2048)], [(0, 4096, 2048)], [(0, 6144, 2048)],
    [(1, 0, 2048)], [(1, 2048, 2048)],
    [(0, 8192, 2048)],
    [(2, 0, 1024), (3, 0, 256), (4, 0, 64)],
    [(0, 10240, 2048)], [(0, 12288, 2048)],
    [(0, 14336, 1024)], [(0, 15360, 1024)],
]


def _block_layout():
    """Per-block column layout, matmul tiles, and the global q-tile table."""
    blocks = []
    q_tiles = []                           # (lvl, level_col, n) per 512-tile
    for segs in BLOCKS:
        w_blk = sum(w for (_, _, w) in segs)
        seg_cols = []
        col = 0
        for (lvl, c0, w) in segs:
            seg_cols.append((lvl, c0, col, w))
            col += w
        mm_tiles = []                      # (block_col, n, lvl, q_index)
        for (lvl, c0, bcol, w) in seg_cols:
            for j in range(0, w, TILE_N):
                n = min(TILE_N, w - j)
                mm_tiles.append((bcol + j, n, lvl, len(q_tiles)))
                q_tiles.append((lvl, c0 + j, n))
        blocks.append(dict(segs=seg_cols, w_blk=w_blk, mm_tiles=mm_tiles))
    return blocks, q_tiles


BLOCK_LAYOUT, Q_TILES = _block_layout()
N_QT = len(Q_TILES)                        # 44
MAX_BW = max(b["w_blk"] for b in BLOCK_LAYOUT)


def _build_module():
    import concourse.mybir as mybir
    from concourse import bacc
    from concourse.tile import TileContext

    dt = mybir.dt
    nc = bacc.Bacc("TRN2", target_bir_lowering=False, debug=False,
                   num_devices=N_CORES)

    fs = [nc.dram_tensor(f"fs{l}", [C, HWS[l]], dt.float32, kind="ExternalInput")
          for l in range(N_LEVELS)]
    ft = [nc.dram_tensor(f"ft{l}", [C, HWS[l]], dt.float32, kind="ExternalInput")
          for l in range(N_LEVELS)]
    # weight chunk ((lvl*2+oc)*2+kc) lives at columns 128+idx*128 (cols 0:128
    # are free for alignment of the early split DMA)
    wt_d = nc.dram_tensor("wt", [128, (N_LEVELS * 4 + 1) * 128], dt.float32,
                          kind="ExternalInput")
    bias_d = nc.dram_tensor("bias", [128, N_LEVELS * 2], dt.float32,
                            kind="ExternalInput")
    out_q_d = nc.dram_tensor("out_q", [1, N_QT * TILE_N], dt.bfloat16,
                             kind="ExternalOutput")

    f32r = dt.float32r
    SUB = mybir.AluOpType.subtract
    SQUARE = mybir.ActivationFunctionType.Square

    with TileContext(nc) as tc:
        with (
            tc.tile_pool(name="const", bufs=1) as const_pool,
            tc.tile_pool(name="feat", bufs=3) as feat_pool,
            tc.tile_pool(name="work", bufs=3) as work_pool,
            tc.tile_pool(name="ps", bufs=6, space="PSUM") as psum_pool,
            tc.tile_pool(name="qps", bufs=2, space="PSUM") as qpsum_pool,
        ):
            wt = const_pool.tile([128, (N_LEVELS * 4 + 1) * 128], f32r)
            bias = const_pool.tile([128, N_LEVELS * 2], dt.float32)
            ones_bf = const_pool.tile([128, 1], dt.bfloat16)
            nc.vector.memset(ones_bf[:], 1.0)
            qcat = const_pool.tile([1, N_QT * TILE_N], dt.bfloat16)
            # zero the partial-tile gap columns (levels 3/4 tiles are
            # narrower) so the final DMA reads initialized memory
            for _qi, (_lvl, _col, _n) in enumerate(Q_TILES):
                if _n < TILE_N:
                    nc.vector.memset(
                        qcat[:, _qi * TILE_N + _n:(_qi + 1) * TILE_N], 0.0)


            # level-0 weights first (small DMA, unblocks the first blocks)
            nc.sync.dma_start(out=wt[:, 0:640],
                              in_=wt_d[:, 0:640].bitcast(f32r))

            def q_phase(mm_tiles, dd0, dd1):
                # column sums over all 256 channels: q = ones^T @ [dd0;dd1]
                for (bcol, n, lvl, qi) in mm_tiles:
                    qps = qpsum_pool.tile([1, TILE_N], dt.float32, tag="qps")
                    nc.tensor.matmul(qps[:, :n], ones_bf[:],
                                     dd0[:, bcol:bcol + n],
                                     start=True, stop=False)
                    nc.tensor.matmul(qps[:, :n], ones_bf[:],
                                     dd1[:, bcol:bcol + n],
                                     start=False, stop=True)
                    nc.scalar.copy(qcat[:, qi * TILE_N:qi * TILE_N + n],
                                   qps[:, :n])

            first = True
            pending = None
            for bi, blk in enumerate(BLOCK_LAYOUT):
                if len(blk["segs"]) > 1:
                    # the fused small-levels block computes slowly; dedicated
                    # right-sized tiles keep it off the main stream's slots
                    bw = blk["w_blk"]
                    s_lo = feat_pool.tile([128, bw], f32r, tag="sm_s_lo",
                                          bufs=1)
                    s_hi = feat_pool.tile([128, bw], f32r, tag="sm_s_hi",
                                          bufs=1)
                    t_lo = feat_pool.tile([128, bw], dt.float32,
                                          tag="sm_t_lo", bufs=1)
                    t_hi = feat_pool.tile([128, bw], dt.float32,
                                          tag="sm_t_hi", bufs=1)
                else:
                    s_lo = feat_pool.tile([128, MAX_BW], f32r, tag="s_lo")
                    s_hi = feat_pool.tile([128, MAX_BW], f32r, tag="s_hi")
                    t_lo = feat_pool.tile([128, MAX_BW], dt.float32,
                                          tag="t_lo")
                    t_hi = feat_pool.tile([128, MAX_BW], dt.float32,
                                          tag="t_hi")
                for (lvl, c0, bcol, w) in blk["segs"]:
                    nc.sync.dma_start(
                        out=s_lo[:, bcol:bcol + w],
                        in_=fs[lvl][0:128, c0:c0 + w].bitcast(f32r))
                    nc.sync.dma_start(
                        out=s_hi[:, bcol:bcol + w],
                        in_=fs[lvl][128:256, c0:c0 + w].bitcast(f32r))
                    nc.sync.dma_start(
                        out=t_lo[:, bcol:bcol + w],
                        in_=ft[lvl][0:128, c0:c0 + w])
                    nc.sync.dma_start(
                        out=t_hi[:, bcol:bcol + w],
                        in_=ft[lvl][128:256, c0:c0 + w])

                if first:
                    nc.sync.dma_start(
                        out=wt[:, 640:(N_LEVELS * 4 + 1) * 128],
                        in_=wt_d[:, 640:(N_LEVELS * 4 + 1) * 128].bitcast(f32r))
                    nc.sync.dma_start(out=bias[:], in_=bias_d[:])
                    first = False

                t_chunks = (t_lo, t_hi)
                dd_ocs = []
                for oc in range(2):
                    d_blk = work_pool.tile([128, MAX_BW], dt.bfloat16, tag="d")
                    for (bcol, n, lvl, qi) in blk["mm_tiles"]:
                        widx = (lvl * 2 + oc) * 2
                        ps = psum_pool.tile([128, TILE_N], dt.float32,
                                            tag="ps")
                        nc.tensor.matmul(
                            ps[:, :n],
                            wt[:, (widx + 1) * 128:(widx + 2) * 128],
                            s_lo[:, bcol:bcol + n],
                            start=True, stop=False)
                        nc.tensor.matmul(
                            ps[:, :n],
                            wt[:, (widx + 2) * 128:(widx + 3) * 128],
                            s_hi[:, bcol:bcol + n],
                            start=False, stop=True)
                        # d = (t - b) - psum; frees the psum bank quickly
                        nc.vector.scalar_tensor_tensor(
                            d_blk[:, bcol:bcol + n],
                            t_chunks[oc][:, bcol:bcol + n],
                            bias[:, lvl * 2 + oc:lvl * 2 + oc + 1],
                            ps[:, :n],
                            op0=SUB, op1=SUB)
                    dd_blk = work_pool.tile([128, MAX_BW], dt.bfloat16,
                                            tag=f"dd{oc}")
                    for (lvl, c0, bcol, w) in blk["segs"]:
                        nc.scalar.activation(
                            dd_blk[:, bcol:bcol + w],
                            d_blk[:, bcol:bcol + w], SQUARE)
                    dd_ocs.append(dd_blk)

                # software-pipelined: emit the PREVIOUS block's q phase so
                # the in-order PE stream never waits on this block's squares
                if pending is not None:
                    q_phase(*pending)
                pending = (blk["mm_tiles"], dd_ocs[0], dd_ocs[1])

            q_phase(*pending)
            nc.sync.dma_start(out=out_q_d[:], in_=qcat[:])

    nc.compile()
    return nc


def _rasterize_masks(gt_bboxes):
    """Host-side mask rasterization, mirroring reference.gt_mask in fp32.

    Returns [B, MASK_LEN] float32 (per-level masks concatenated)."""
    out = np.zeros((B, MASK_LEN), np.float32)
    for lvl in range(N_LEVELS):
        h = w = SIZES[lvl]
        stride = np.float32(STRIDES[lvl])
        off = MASK_OFF[lvl]
        q = np.floor(gt_bboxes.astype(np.float32) / stride).astype(np.int32)
        lx = np.minimum(q[..., 0], w - 1)
        ly = np.minimum(q[..., 1], h - 1)
        rx = np.minimum(q[..., 2], w - 1)
        ry = np.minimum(q[..., 3], h - 1)
        for b in range(B):
            m = np.zeros((h, w), bool)
            for i in range(gt_bboxes.shape[1]):
                if lx[b, i] == rx[b, i] or ly[b, i] == ry[b, i]:
                    m[ly[b, i], lx[b, i]] = True
                else:
                    m[ly[b, i]:ry[b, i], lx[b, i]:rx[b, i]] = True
            out[b, off:off + h * w] = m.reshape(-1).astype(np.float32)
    return out


_NC_CACHE = None


def _get_nc():
    global _NC_CACHE
    if _NC_CACHE is None:
        _NC_CACHE = _build_module()
    return _NC_CACHE


def _run(in_maps, trace=False, trace_cores=None):
    from concourse.bass_utils import run_bass_kernel_spmd

    kwargs = {}
    if trace:
        kwargs.update(trace=True, trace_cores=trace_cores or [0])
    return run_bass_kernel_spmd(_get_nc(), in_maps, core_ids=list(range(N_CORES)),
                                **kwargs)


def _pack_const(inputs):
    """Pack replicated weights/bias: chunk ((lvl*2+oc)*2+kc) at 128+idx*128
    holds w_lvl[oc*128+o_local, kc*128+c_local] transposed."""
    wt_packed = np.zeros((128, (N_LEVELS * 4 + 1) * 128), np.float32)
    bias_packed = np.zeros((128, N_LEVELS * 2), np.float32)
    for lvl in range(N_LEVELS):
        w = np.asarray(inputs[f"adapt_w{lvl}"], np.float32)
        bvec = np.asarray(inputs[f"adapt_b{lvl}"], np.float32)
        for oc in range(2):
            bias_packed[:, lvl * 2 + oc] = bvec[oc * 128:(oc + 1) * 128]
            for kc in range(2):
                idx = (lvl * 2 + oc) * 2 + kc
                blk = w[oc * 128:(oc + 1) * 128, kc * 128:(kc + 1) * 128]
                wt_packed[:, 128 + idx * 128:128 + (idx + 1) * 128] = blk.T
    return wt_packed, bias_packed


def kernel(_trace=False, _return_results=False, **inputs):
    gt_bboxes = np.asarray(inputs["gt_bboxes"], np.float32)
    masks = _rasterize_masks(gt_bboxes)
    wt_packed, bias_packed = _pack_const(inputs)

    in_maps = []
    for b in range(N_CORES):
        m = {"wt": wt_packed, "bias": bias_packed}
        for lvl in range(N_LEVELS):
            m[f"fs{lvl}"] = np.ascontiguousarray(
                np.asarray(inputs[f"feat_s{lvl}"][b], np.float32).reshape(C, HWS[lvl]))
            m[f"ft{lvl}"] = np.ascontiguousarray(
                np.asarray(inputs[f"feat_t{lvl}"][b], np.float32).reshape(C, HWS[lvl]))
        in_maps.append(m)

    res = _run(in_maps, trace=_trace)

    s_tot = np.zeros(N_LEVELS, np.float64)
    s_gt = np.zeros(N_LEVELS, np.float64)
    for c in range(N_CORES):
        q = res.results[c]["out_q"].astype(np.float64).reshape(-1)
        for qi, (lvl, col, n) in enumerate(Q_TILES):
            qv = q[qi * TILE_N:qi * TILE_N + n]
            mv = masks[c, MASK_OFF[lvl] + col:MASK_OFF[lvl] + col + n].astype(np.float64)
            s_tot[lvl] += qv.sum()
            s_gt[lvl] += (qv * mv).sum()

    loss = np.float64(0.0)
    for lvl in range(N_LEVELS):
        s_bg = s_tot[lvl] - s_gt[lvl]
        loss += WEIGHT_GT * np.sqrt(s_gt[lvl] + 1e-8) + \
            WEIGHT_BG * np.sqrt(s_bg + 1e-8)

    out = np.array(loss, dtype=np.float32)
    if _return_results:
        return out, res
    return out

